# revision 6
# baseline (speedup 1.0000x reference)
"""Bass/Trainium2 kernel for nn_AvgPoolBackbone (segment_reduce).

Computes, for each batch row b of x [B, S, D]:
    eff = S if idx[b] == -1 else idx[b]
    out[b] = mean(x[b, :eff], axis=0)   (zeros when eff <= 0)

Strategy
--------
Pure data parallel over 8 NeuronCores (16 batches each).  On the host we
fold the prefix mask AND the 1/eff_len scaling into a single f32 matrix
`maskt` so the device does no division and no control flow.

Per batch, x[b] ([2048, 256] f32, 2 MiB) is viewed as [128, 16*256]:
partition p holds the 16 consecutive sequence rows p*16..p*16+15 — one
contiguous 16 KiB DRAM run per partition, which keeps the DMA descriptors
large.  The masked mean is then 16 PSUM-accumulated TensorE matmuls

    psum[1, D] += maskt[:, col].T @ x_view[:, j*D:(j+1)*D]

where maskt[p, col] = mask[b, p*16 + j] / eff_len[b].  Operands are
bitcast to float32r, which streams one PSUM row per cycle (4x faster
than the two-pass fp32 path) at N=256.  TensorE does the masking and the
cross-partition reduction in one instruction; the kernel is
HBM-bandwidth bound.
"""

import numpy as np

import concourse.bass as bass
import concourse.tile as tile
from concourse import bacc, mybir
from concourse import bass_utils

F32 = mybir.dt.float32
F32R = mybir.dt.float32r

# Problem config (hardcoded per the harness contract).
B, S, D = 128, 2048, 256
N_CORES = 8
BL = B // N_CORES  # batches per core
P = 128            # SBUF partitions
CHUNK_B = 1        # batches loaded per DMA


def build_kernel(bl=BL, s=S, d=D, chunk_b=CHUNK_B, f32r=False, q_dve=9, bufs=6):
    """Build + compile the single-core Bass module (same NEFF on all cores).

    Per batch the 16 j-slices are split: the first `q_dve` go through a
    DVE fused multiply-accumulate chain (exact fp32), the rest through
    PE matmuls; one final ones-matmul folds the DVE partial into the
    same PSUM accumulation group.  With f32r=True everything instead
    runs on PE in the reduced-precision float32r format (faster PE, but
    ~1.5e-4 rel err); q_dve is ignored then.
    """
    j = s // P  # seq rows per partition (16 at full size)
    mmdt = F32R if f32r else F32
    if f32r:
        q_dve = 0
    assert 0 <= q_dve < j
    nc = bacc.Bacc("TRN2", target_bir_lowering=False, debug=False)
    x = nc.dram_tensor("x", (bl, s, d), mmdt, kind="ExternalInput")
    maskt = nc.dram_tensor("maskt", (P, bl * j), mmdt, kind="ExternalInput")
    out = nc.dram_tensor("out", (1, bl * d), F32, kind="ExternalOutput")

    with tile.TileContext(nc) as tc:
        with (
            tc.tile_pool(name="xp", bufs=bufs) as xp,
            tc.tile_pool(name="mp", bufs=1) as mp,
            tc.tile_pool(name="op", bufs=1) as op,
            tc.tile_pool(name="ap", bufs=4) as apool,
            tc.tile_pool(name="ps", bufs=8, space=bass.MemorySpace.PSUM) as ps,
        ):
            m_t = mp.tile([P, bl * j], mmdt)
            nc.sync.dma_start(m_t[:], maskt.ap())
            ones_t = None
            if q_dve > 0:
                ones_t = mp.tile([P, 1], F32)
                nc.vector.memset(ones_t[:], 1.0)
            o_t = op.tile([1, bl * d], F32)
            for ci, b0 in enumerate(range(0, bl, chunk_b)):
                nb = min(chunk_b, bl - b0)
                x_t = xp.tile([P, nb, j * d], mmdt)
                # sbuf[p, b, ji*d + di] = x[b0+b, p*j + ji, di]
                # -> per (p, b) one contiguous j*d*4-byte DRAM run
                dma_eng = nc.sync if ci % 2 == 0 else nc.scalar
                dma_eng.dma_start(
                    x_t[:],
                    x.ap()[b0 : b0 + nb].rearrange("b (p j) d -> p b (j d)", p=P),
                )
                for bi in range(nb):
                    b = b0 + bi
                    acc = ps.tile([1, d], F32)
                    if q_dve > 0:
                        acc_sb = apool.tile([P, d], F32)
                        for ji in range(q_dve):
                            col = b * j + ji
                            xs = x_t[:, bi, ji * d : (ji + 1) * d]
                            mcol = m_t[:, col : col + 1]
                            if ji == 0:
                                nc.vector.tensor_scalar_mul(acc_sb[:], xs, mcol)
                            else:
                                nc.vector.scalar_tensor_tensor(
                                    acc_sb[:],
                                    xs,
                                    mcol,
                                    acc_sb[:],
                                    mybir.AluOpType.mult,
                                    mybir.AluOpType.add,
                                )
                    for ji in range(q_dve, j):
                        col = b * j + ji
                        nc.tensor.matmul(
                            acc[:],
                            m_t[:, col : col + 1],
                            x_t[:, bi, ji * d : (ji + 1) * d],
                            start=(ji == q_dve),
                            stop=(ji == j - 1 and q_dve == 0),
                        )
                    if q_dve > 0:
                        # fold the DVE partial sums into the same PSUM group
                        nc.tensor.matmul(
                            acc[:],
                            ones_t[:],
                            acc_sb[:],
                            start=False,
                            stop=True,
                        )
                    nc.vector.tensor_copy(o_t[:, b * d : (b + 1) * d], acc[:])
            nc.sync.dma_start(out.ap(), o_t[:])

    nc.compile()
    return nc


def make_host_inputs(x, start_padding_indices, n_cores=N_CORES, bl=BL, s=S, d=D):
    """Shard x and build the per-core scaled mask matrices.

    maskt[p, b*j + ji] = (p*j + ji < eff[b]) / max(eff[b], 1)
    """
    x = np.ascontiguousarray(np.asarray(x, dtype=np.float32))
    idx = np.asarray(start_padding_indices).astype(np.int64)
    j = s // P
    eff = np.where(idx == -1, s, idx).astype(np.int64)  # [B]
    scale = 1.0 / np.maximum(eff, 1).astype(np.float64)
    mask = (np.arange(s)[None, :] < eff[:, None]) * scale[:, None]  # [B, S] f64
    mask = mask.astype(np.float32)
    # [B, S] -> [B, P, j] (s-major within partition) -> cores pack [P, bl*j]
    mask_pj = mask.reshape(-1, P, j)  # [B, P, j]
    in_maps = []
    for c in range(n_cores):
        mb = mask_pj[c * bl : (c + 1) * bl]  # [bl, P, j]
        maskt = np.ascontiguousarray(mb.transpose(1, 0, 2).reshape(P, bl * j))
        in_maps.append(
            {
                "x": np.ascontiguousarray(x[c * bl : (c + 1) * bl]),
                "maskt": maskt,
            }
        )
    return in_maps


_CACHED_NC = None


def _get_nc():
    global _CACHED_NC
    if _CACHED_NC is None:
        _CACHED_NC = build_kernel()
    return _CACHED_NC


def run(x, start_padding_indices, trace=False):
    """Run on all 8 cores; returns (out [B, D] f32, BassKernelResults)."""
    nc = _get_nc()
    in_maps = make_host_inputs(x, start_padding_indices)
    res = bass_utils.run_bass_kernel_spmd(
        nc, in_maps, core_ids=list(range(N_CORES)), trace=trace
    )
    outs = [r["out"].reshape(BL, D) for r in res.results]
    return np.concatenate(outs, axis=0), res


def kernel(x, start_padding_indices):
    out, _ = run(x, start_padding_indices, trace=False)
    return out


# revision 8
# speedup vs baseline: 1.0947x; 1.0947x over previous
"""Bass/Trainium2 kernel for nn_AvgPoolBackbone (segment_reduce).

Computes, for each batch row b of x [B, S, D]:
    eff = S if idx[b] == -1 else idx[b]
    out[b] = mean(x[b, :eff], axis=0)   (zeros when eff <= 0)

Strategy
--------
Pure data parallel over 8 NeuronCores (16 batches each).  On the host we
fold the prefix mask AND the 1/eff_len scaling into a single f32 matrix
`maskt` so the device does no division and no control flow.

Per batch, x[b] ([2048, 256] f32, 2 MiB) is viewed as [128, 16*256]:
partition p holds the 16 consecutive sequence rows p*16..p*16+15 — one
contiguous 16 KiB DRAM run per partition, which keeps the DMA descriptors
large.  The masked mean is then 16 PSUM-accumulated TensorE matmuls

    psum[1, D] += maskt[:, col].T @ x_view[:, j*D:(j+1)*D]

where maskt[p, col] = mask[b, p*16 + j] / eff_len[b].  Operands are
bitcast to float32r, which streams one PSUM row per cycle (4x faster
than the two-pass fp32 path) at N=256.  TensorE does the masking and the
cross-partition reduction in one instruction; the kernel is
HBM-bandwidth bound.
"""

import numpy as np

import concourse.bass as bass
import concourse.tile as tile
from concourse import bacc, mybir
from concourse import bass_utils

F32 = mybir.dt.float32
F32R = mybir.dt.float32r

# Problem config (hardcoded per the harness contract).
B, S, D = 128, 2048, 256
N_CORES = 8
BL = B // N_CORES  # batches per core
P = 128            # SBUF partitions
CHUNK_B = 2        # batches loaded per DMA


def build_kernel(bl=BL, s=S, d=D, chunk_b=CHUNK_B, f32r=False, dve_mod=2, bufs=3):
    """Build + compile the single-core Bass module (same NEFF on all cores).

    Batches alternate between two engines to halve the per-engine load
    while keeping exact fp32: batches with b % dve_mod == 0 run a DVE
    fused multiply-accumulate chain (then one PE ones-matmul folds the
    [128, d] partials across partitions); the other batches run 16
    PSUM-accumulated PE matmuls.  PSUM->SBUF result copies go to the
    otherwise idle ScalarE.  With f32r=True everything instead runs on
    PE in reduced-precision float32r (dve_mod ignored).
    """
    j = s // P  # seq rows per partition (16 at full size)
    mmdt = F32R if f32r else F32
    if f32r:
        dve_mod = 0
    nc = bacc.Bacc("TRN2", target_bir_lowering=False, debug=False)
    x = nc.dram_tensor("x", (bl, s, d), mmdt, kind="ExternalInput")
    maskt = nc.dram_tensor("maskt", (P, bl * j), mmdt, kind="ExternalInput")
    out = nc.dram_tensor("out", (1, bl * d), F32, kind="ExternalOutput")

    def is_dve(b):
        return dve_mod > 0 and b % dve_mod == 0

    with tile.TileContext(nc) as tc:
        with (
            tc.tile_pool(name="xp", bufs=bufs) as xp,
            tc.tile_pool(name="mp", bufs=1) as mp,
            tc.tile_pool(name="op", bufs=1) as op,
            tc.tile_pool(name="ap", bufs=3) as apool,
            tc.tile_pool(name="ps", bufs=8, space=bass.MemorySpace.PSUM) as ps,
        ):
            m_t = mp.tile([P, bl * j], mmdt)
            nc.sync.dma_start(m_t[:], maskt.ap())
            ones_t = None
            if dve_mod > 0:
                ones_t = mp.tile([P, 1], F32)
                nc.vector.memset(ones_t[:], 1.0)
            o_t = op.tile([1, bl * d], F32)
            for ci, b0 in enumerate(range(0, bl, chunk_b)):
                nb = min(chunk_b, bl - b0)
                x_t = xp.tile([P, nb, j * d], mmdt)
                # sbuf[p, b, ji*d + di] = x[b0+b, p*j + ji, di]
                # -> per (p, b) one contiguous j*d*4-byte DRAM run
                dma_eng = nc.sync if ci % 2 == 0 else nc.scalar
                dma_eng.dma_start(
                    x_t[:],
                    x.ap()[b0 : b0 + nb].rearrange("b (p j) d -> p b (j d)", p=P),
                )
                # DVE chains first so PE's fold-matmul (emitted after the
                # PE batches) finds its input ready.
                accs = {}
                for bi in range(nb):
                    if not is_dve(b0 + bi):
                        continue
                    b = b0 + bi
                    acc_sb = apool.tile([P, d], F32)
                    accs[bi] = acc_sb
                    for ji in range(j):
                        col = b * j + ji
                        xs = x_t[:, bi, ji * d : (ji + 1) * d]
                        mcol = m_t[:, col : col + 1]
                        if ji == 0:
                            nc.vector.tensor_scalar_mul(acc_sb[:], xs, mcol)
                        else:
                            nc.vector.scalar_tensor_tensor(
                                acc_sb[:],
                                xs,
                                mcol,
                                acc_sb[:],
                                mybir.AluOpType.mult,
                                mybir.AluOpType.add,
                            )
                for bi in range(nb):
                    if is_dve(b0 + bi):
                        continue
                    b = b0 + bi
                    acc = ps.tile([1, d], F32)
                    for ji in range(j):
                        col = b * j + ji
                        nc.tensor.matmul(
                            acc[:],
                            m_t[:, col : col + 1],
                            x_t[:, bi, ji * d : (ji + 1) * d],
                            start=(ji == 0),
                            stop=(ji == j - 1),
                        )
                    nc.scalar.copy(o_t[:, b * d : (b + 1) * d], acc[:])
                for bi, acc_sb in accs.items():
                    b = b0 + bi
                    acc = ps.tile([1, d], F32)
                    nc.tensor.matmul(
                        acc[:], ones_t[:], acc_sb[:], start=True, stop=True
                    )
                    nc.scalar.copy(o_t[:, b * d : (b + 1) * d], acc[:])
            nc.sync.dma_start(out.ap(), o_t[:])

    nc.compile()
    return nc


def make_host_inputs(x, start_padding_indices, n_cores=N_CORES, bl=BL, s=S, d=D):
    """Shard x and build the per-core scaled mask matrices.

    maskt[p, b*j + ji] = (p*j + ji < eff[b]) / max(eff[b], 1)
    """
    x = np.ascontiguousarray(np.asarray(x, dtype=np.float32))
    idx = np.asarray(start_padding_indices).astype(np.int64)
    j = s // P
    eff = np.where(idx == -1, s, idx).astype(np.int64)  # [B]
    scale = 1.0 / np.maximum(eff, 1).astype(np.float64)
    mask = (np.arange(s)[None, :] < eff[:, None]) * scale[:, None]  # [B, S] f64
    mask = mask.astype(np.float32)
    # [B, S] -> [B, P, j] (s-major within partition) -> cores pack [P, bl*j]
    mask_pj = mask.reshape(-1, P, j)  # [B, P, j]
    in_maps = []
    for c in range(n_cores):
        mb = mask_pj[c * bl : (c + 1) * bl]  # [bl, P, j]
        maskt = np.ascontiguousarray(mb.transpose(1, 0, 2).reshape(P, bl * j))
        in_maps.append(
            {
                "x": np.ascontiguousarray(x[c * bl : (c + 1) * bl]),
                "maskt": maskt,
            }
        )
    return in_maps


_CACHED_NC = None


def _get_nc():
    global _CACHED_NC
    if _CACHED_NC is None:
        _CACHED_NC = build_kernel()
    return _CACHED_NC


def run(x, start_padding_indices, trace=False):
    """Run on all 8 cores; returns (out [B, D] f32, BassKernelResults)."""
    nc = _get_nc()
    in_maps = make_host_inputs(x, start_padding_indices)
    res = bass_utils.run_bass_kernel_spmd(
        nc, in_maps, core_ids=list(range(N_CORES)), trace=trace
    )
    outs = [r["out"].reshape(BL, D) for r in res.results]
    return np.concatenate(outs, axis=0), res


def kernel(x, start_padding_indices):
    out, _ = run(x, start_padding_indices, trace=False)
    return out
